# revision 7
# baseline (speedup 1.0000x reference)
"""Trainium2 Bass kernel for nn_BatchContrastLoss (InfoNCE-style contrastive loss).

Reference computation:
    sim[i,j]  = cos(que_i, ans_j)            (eps-guarded norms)
    logits    = sim / 0.07
    loss      = -mean_i(log_softmax(logits, axis=1)[i,i])

Sharding: data-parallel over rows of que across 8 NeuronCores. Each core
computes its [512, 4096] logits slab against the full ans batch and reduces
each row to a softmax denominator sum_j exp(logits[i,j]). The host takes
log + mean and subtracts the diagonal (the "all-reduce" of the hint).

Key design decisions (v2 — was 101us, DVE/ScalarE-bound):
  - Row norms are folded into the fp8 quantization on the host: rows are
    normalized to unit length, scaled by 16 (keeps e4m3 mantissa well fed;
    entries ~N(0, 0.5)), and quantized. The device then needs NO norm
    computation at all: psum = (16*qhat)·(16*ahat) = 256*cos, and the exp
    drain folds 1/(256*gamma) into its free affine scale. This removes every
    DVE instruction and all ones-matmul norm reductions from the v1 kernel
    (DVE was 64us busy, ScalarE 64us, and the PE sat idle 27us waiting).
  - The diagonal logits_ii are computed exactly on the host in f32 (O(B*D),
    negligible) — only the O(B^2*D) denominator work runs on device.
  - fp8e4m3 DoubleRow matmuls: K=256 per instruction, N=512 moving columns,
    measured 216ns issue-to-issue warm => 128 MMs ~ 27.6us/core floor.
  - Weight-stationary-ish order (g: 1024-col group, m: 128-row tile,
    c: 512-col bank, t: 256-d k-pair): one [128,1024] 2-bank PSUM tile per
    (g,m), drained by a single ScalarE Exp with accum_out row-sums
    ((1024+352)/1.2 ~ 1.15us each, 16 total => ScalarE ~60% busy, off the
    critical path).
  - DMA: ans arrives as 4 x 1MB groups (>=1MB transfers run near peak), in
    consumption order; group 0 is split per-k-pair so the first matmul can
    start after ~256KB. que (512KB) rides the second HWDGE ring (ScalarE).
  - The PE clock gate (HAM) needs ~3.4us of sustained activity to unthrottle
    from 1.2 to 2.4 GHz; N_WARM dummy matmuls on a zeroed scratch tile fill
    the DMA lead-in so the real matmuls run warm from the start. A dummy Exp
    on ScalarE pulls the ~2.7us activation table load off the critical path.
"""

import numpy as np

import concourse.bass as bass
import concourse.mybir as mybir
import concourse.tile as tile
from concourse import bacc
from concourse.bass_utils import run_bass_kernel_spmd

# Problem constants (self-contained; the harness provides only the inputs).
B = 4096  # rows of que_batch / ans_batch
D = 1024  # feature dim
NCORES = 8
NB = B // NCORES  # local que rows per core = 512
P = 128  # SBUF partitions
KT2 = 4  # k-pair tiles (each DoubleRow matmul contracts 256 dims)
NW = 512  # matmul moving width = one fp32 PSUM bank
G = 4  # ans column groups of 1024
MT = NB // P  # 4 row tiles of 128
GAMA = 0.07
EPS = 1e-8
SCALE = 16.0  # host quantization scale on unit rows
EXP_SCALE = 1.0 / (SCALE * SCALE * GAMA)  # psum -> logits
N_WARM = 10  # dummy matmuls to unthrottle the PE clock during DMA fill

F32 = mybir.dt.float32
FP8 = mybir.dt.float8e4  # e4m3
DR = mybir.MatmulPerfMode.DoubleRow
AF = mybir.ActivationFunctionType

OUTPUT_NAMES = ["s_out"]


def _build_program():
    nc = bacc.Bacc(
        "TRN2", target_bir_lowering=False, debug=False, num_devices=NCORES
    )

    # qPK[p, 2t+i, m] = q16hat_fp8[local row m, d=256t+128i+p]
    qPK = nc.dram_tensor("qPK", [P, 2 * KT2, NB], FP8, kind="ExternalInput").ap()
    # aPK[g, p, 2t+i, j] = a16hat_fp8[col 1024g+j, d=256t+128i+p]
    aPK = nc.dram_tensor("aPK", [G, P, 2 * KT2, 1024], FP8, kind="ExternalInput").ap()
    # s_out[p, 4g+m] = sum_{j in group g} exp(logits[row 128m+p, j]);
    # the last slab (g=3,m=3) is split into cols 15+16 (per-bank drains).
    s_out = nc.dram_tensor("s_out", [P, G * MT + 1], F32, kind="ExternalOutput").ap()

    with tile.TileContext(nc) as tc:
        with (
            tc.tile_pool(name="persist", bufs=1) as persist,
            tc.tile_pool(name="psp", bufs=4, space="PSUM") as psp,
        ):
            _body(nc, persist, psp, qPK, aPK, s_out)

    nc.compile()
    return nc


def _body(nc, persist, psp, qPK, aPK, s_out):
    # ---- DMA front, all on the SP HWDGE ring in strict consumption order
    # (the two HWDGE rings share the 16 SDMA engines, so splitting across
    # rings just makes the first transfer slower). >=512KB transfers run at
    # ~350-400GB/s; small ones are descriptor/latency-dominated, so ship
    # whole 1MB column groups.
    qall = persist.tile([P, 2 * KT2, NB], FP8, tag="qall")
    nc.sync.dma_start(out=qall, in_=qPK)
    ags = []
    for g in range(G):
        a = persist.tile([P, 2 * KT2, 1024], FP8, tag=f"ag_{g}", name=f"ag_{g}")
        nc.sync.dma_start(out=a, in_=aPK[g])
        ags.append(a)

    # ---- warmup: dummy Exp triggers the one-time activation table load;
    # dummy DoubleRow matmuls keep the PE busy through the HAM window so the
    # real matmuls run at 2.4 GHz as soon as their data lands. N=256 keeps
    # per-dummy granularity fine (~320ns cold) so the overshoot past DMA
    # arrival is small. All on zeroed scratch, off to the side.
    scr8 = persist.tile([P, 2, 256], FP8, tag="scr8")
    nc.gpsimd.memset(scr8, 0.0)
    scrf = persist.tile([P, 1], F32, tag="scrf")
    nc.gpsimd.memset(scrf, 0.0)
    dumo = persist.tile([P, 1], F32, tag="dumo")
    nc.scalar.activation(dumo, scrf, AF.Exp)

    ppw = psp.tile([P, 2 * NW], F32, tag="pp", name="pp_warm")
    for w in range(N_WARM):
        nc.tensor.matmul(
            ppw[:, 0:256],
            lhsT=scr8[:, :, 0:P],
            rhs=scr8,
            start=True,
            stop=True,
            perf_mode=DR,
        )

    # ---- main loop: 16 (g, m) slabs of [128 rows x 1024 cols], each one
    # 2-bank PSUM tile built by 8 DoubleRow matmuls, drained in-place by a
    # single Exp with fused row-sum accumulation. The very last slab is
    # drained per-bank (two 512-col Exps) so the post-matmul tail is shorter,
    # and the first 12 accumulator columns are shipped out early so the final
    # DMA is tiny.
    s_sb_a = persist.tile([P, 12], F32, tag="s_sb_a")
    s_sb_b = persist.tile([P, 5], F32, tag="s_sb_b")
    for g in range(G):
        for m in range(MT):
            pp = psp.tile([P, 2 * NW], F32, tag="pp", name=f"pp_{g}_{m}")
            last = g == G - 1 and m == MT - 1
            for c in range(2):
                for t in range(KT2):
                    rhs = ags[g][:, 2 * t : 2 * t + 2, c * NW : (c + 1) * NW]
                    nc.tensor.matmul(
                        pp[:, c * NW : (c + 1) * NW],
                        lhsT=qall[:, 2 * t : 2 * t + 2, m * P : (m + 1) * P],
                        rhs=rhs,
                        start=(t == 0),
                        stop=(t == KT2 - 1),
                        perf_mode=DR,
                    )
                if last:
                    nc.scalar.activation(
                        pp[:, c * NW : (c + 1) * NW],
                        pp[:, c * NW : (c + 1) * NW],
                        AF.Exp,
                        scale=float(EXP_SCALE),
                        accum_out=s_sb_b[:, 3 + c : 4 + c],
                    )
            if not last:
                col = g * MT + m
                acc = (
                    s_sb_a[:, col : col + 1]
                    if col < 12
                    else s_sb_b[:, col - 12 : col - 11]
                )
                nc.scalar.activation(
                    pp,
                    pp,
                    AF.Exp,
                    scale=float(EXP_SCALE),
                    accum_out=acc,
                )
        if g == G - 2:
            nc.sync.dma_start(out=s_out[:, 0:12], in_=s_sb_a)

    nc.sync.dma_start(out=s_out[:, 12:17], in_=s_sb_b)


_CACHE = {}


def _get_program():
    if "nc" not in _CACHE:
        _CACHE["nc"] = _build_program()
    return _CACHE["nc"]


def _make_in_maps(que, ans):
    """Normalize rows (folding the cosine norms into the quantization scale),
    quantize to fp8e4m3, and pack into the on-chip tile layouts. Also returns
    the exact host-computed diagonal logits."""
    fp8 = mybir.dt.np(FP8)
    que = np.asarray(que, dtype=np.float32)
    ans = np.asarray(ans, dtype=np.float32)

    qn = np.maximum(np.sqrt((que.astype(np.float64) ** 2).sum(1)), EPS)
    an = np.maximum(np.sqrt((ans.astype(np.float64) ** 2).sum(1)), EPS)
    q8 = (que * (SCALE / qn[:, None]).astype(np.float32)).astype(fp8)
    a8 = (ans * (SCALE / an[:, None]).astype(np.float32)).astype(fp8)

    # diag logits (exact, f64): cos(q_i, a_i) / gamma
    diag = (que.astype(np.float64) * ans.astype(np.float64)).sum(1) / (
        qn * an * GAMA
    )

    # aPK[g, p, 2t+i, j] = a8[1024g+j, 256t+128i+p]  (shared by all cores)
    aPK = np.ascontiguousarray(
        a8.reshape(G, 1024, KT2, 2, P).transpose(0, 4, 2, 3, 1)
    ).reshape(G, P, 2 * KT2, 1024)

    in_maps = []
    for c in range(NCORES):
        qc = q8[c * NB : (c + 1) * NB]  # [512, 1024]
        qPK = np.ascontiguousarray(
            qc.reshape(NB, KT2, 2, P).transpose(3, 1, 2, 0)
        ).reshape(P, 2 * KT2, NB)
        in_maps.append({"qPK": qPK, "aPK": aPK})
    return in_maps, diag


def _finish(results, diag):
    # s_out[p, 4g+m]: per-group partial softmax denominators (cols 15+16 are
    # the two halves of the last slab).
    denoms = []
    for r in results:
        so = np.asarray(r["s_out"])  # [P, 17]
        s16 = np.concatenate([so[:, :15], (so[:, 15] + so[:, 16])[:, None]], axis=1)
        s = s16.reshape(P, G, MT).sum(axis=1)  # [p, m]
        denoms.append(s.T.reshape(-1))  # local row order m*128+p
    denom = np.concatenate(denoms)  # [B]
    lse = np.log(denom.astype(np.float64))
    loss = np.float32(np.mean(lse - diag))
    return np.array([loss], dtype=np.float32)


def kernel(que_batch, ans_batch):
    nc = _get_program()
    in_maps, diag = _make_in_maps(np.asarray(que_batch), np.asarray(ans_batch))
    res = run_bass_kernel_spmd(nc, in_maps, list(range(NCORES)))
    return _finish(res.results, diag)


if __name__ == "__main__":
    rng = np.random.default_rng(0)
    q = rng.standard_normal((B, D), dtype=np.float32)
    a = rng.standard_normal((B, D), dtype=np.float32)
    print(kernel(q, a))


# revision 14
# speedup vs baseline: 1.0464x; 1.0464x over previous
"""Trainium2 Bass kernel for nn_BatchContrastLoss (InfoNCE-style contrastive loss).

Reference computation:
    sim[i,j]  = cos(que_i, ans_j)            (eps-guarded norms)
    logits    = sim / 0.07
    loss      = -mean_i(log_softmax(logits, axis=1)[i,i])

Sharding: data-parallel over rows of que across 8 NeuronCores. Each core
computes its [512, 4096] logits slab against the full ans batch and reduces
each row to a softmax denominator sum_j exp(logits[i,j]). The host takes
log + mean and subtracts the diagonal (the "all-reduce" of the hint).

Key design decisions (v2 — was 101us, DVE/ScalarE-bound):
  - Row norms are folded into the fp8 quantization on the host: rows are
    normalized to unit length, scaled by 16 (keeps e4m3 mantissa well fed;
    entries ~N(0, 0.5)), and quantized. The device then needs NO norm
    computation at all: psum = (16*qhat)·(16*ahat) = 256*cos, and the exp
    drain folds 1/(256*gamma) into its free affine scale. This removes every
    DVE instruction and all ones-matmul norm reductions from the v1 kernel
    (DVE was 64us busy, ScalarE 64us, and the PE sat idle 27us waiting).
  - The diagonal logits_ii are computed exactly on the host in f32 (O(B*D),
    negligible) — only the O(B^2*D) denominator work runs on device.
  - fp8e4m3 DoubleRow matmuls: K=256 per instruction, N=512 moving columns,
    measured 216ns issue-to-issue warm => 128 MMs ~ 27.6us/core floor.
  - Weight-stationary-ish order (g: 1024-col group, m: 128-row tile,
    c: 512-col bank, t: 256-d k-pair): one [128,1024] 2-bank PSUM tile per
    (g,m), drained by a single ScalarE Exp with accum_out row-sums
    ((1024+352)/1.2 ~ 1.15us each, 16 total => ScalarE ~60% busy, off the
    critical path).
  - DMA: ans arrives as 4 x 1MB groups (>=1MB transfers run near peak), in
    consumption order; group 0 is split per-k-pair so the first matmul can
    start after ~256KB. que (512KB) rides the second HWDGE ring (ScalarE).
  - The PE clock gate (HAM) needs ~3.4us of sustained activity to unthrottle
    from 1.2 to 2.4 GHz; N_WARM dummy matmuls on a zeroed scratch tile fill
    the DMA lead-in so the real matmuls run warm from the start. A dummy Exp
    on ScalarE pulls the ~2.7us activation table load off the critical path.
"""

import numpy as np

import concourse.bass as bass
import concourse.mybir as mybir
import concourse.tile as tile
from concourse import bacc
from concourse.bass_utils import run_bass_kernel_spmd

# Problem constants (self-contained; the harness provides only the inputs).
B = 4096  # rows of que_batch / ans_batch
D = 1024  # feature dim
NCORES = 8
NB = B // NCORES  # local que rows per core = 512
P = 128  # SBUF partitions
KT2 = 4  # k-pair tiles (each DoubleRow matmul contracts 256 dims)
NW = 512  # matmul moving width = one fp32 PSUM bank
G = 4  # ans column groups of 1024
MT = NB // P  # 4 row tiles of 128
GAMA = 0.07
EPS = 1e-8
SCALE = 16.0  # host quantization scale on unit rows
EXP_SCALE = 1.0 / (SCALE * SCALE * GAMA)  # psum -> logits
N_WARM = 16  # dummy matmuls to unthrottle the PE clock during DMA fill

F32 = mybir.dt.float32
FP8 = mybir.dt.float8e4  # e4m3
DR = mybir.MatmulPerfMode.DoubleRow
AF = mybir.ActivationFunctionType

OUTPUT_NAMES = ["s_out"]


def _build_program():
    nc = bacc.Bacc(
        "TRN2", target_bir_lowering=False, debug=False, num_devices=NCORES
    )

    # qPK[m, p, 2t+i, mm] = q16hat_fp8[local row 128m+mm, d=256t+128i+p]
    qPK = nc.dram_tensor("qPK", [MT, P, 2 * KT2, P], FP8, kind="ExternalInput").ap()
    # aPK[g, p, 2t+i, j] = a16hat_fp8[col 1024g+j, d=256t+128i+p]
    aPK = nc.dram_tensor("aPK", [G, P, 2 * KT2, 1024], FP8, kind="ExternalInput").ap()
    # s_out[p, 4g+m] = sum_{j in group g} exp(logits[row 128m+p, j])
    s_out = nc.dram_tensor("s_out", [P, G * MT], F32, kind="ExternalOutput").ap()

    with tile.TileContext(nc) as tc:
        with (
            tc.tile_pool(name="persist", bufs=1) as persist,
            tc.tile_pool(name="psp", bufs=4, space="PSUM") as psp,
        ):
            _body(nc, persist, psp, qPK, aPK, s_out)

    nc.compile()
    return nc


def _body(nc, persist, psp, qPK, aPK, s_out):
    # ---- DMA front, all on the SP HWDGE ring in strict consumption order
    # (the two HWDGE rings share the 16 SDMA engines, so splitting across
    # rings just makes the first transfer slower). The first real matmul is
    # gated on qm[0] (128KB) + ag[0] (1MB) only; everything else streams
    # behind while earlier slabs compute.
    qms = []
    ags = []

    def dma_q(m):
        qm = persist.tile([P, 2 * KT2, P], FP8, tag=f"qm_{m}", name=f"qm_{m}")
        nc.sync.dma_start(out=qm, in_=qPK[m])
        qms.append(qm)

    def dma_a(g):
        a = persist.tile([P, 2 * KT2, 1024], FP8, tag=f"ag_{g}", name=f"ag_{g}")
        nc.sync.dma_start(out=a, in_=aPK[g])
        ags.append(a)

    dma_q(0)
    dma_a(0)
    for m in range(1, MT):
        dma_q(m)
    for g in range(1, G):
        dma_a(g)

    # ---- warmup: dummy Exp triggers the one-time activation table load;
    # dummy DoubleRow matmuls keep the PE busy through the HAM window so the
    # real matmuls run at 2.4 GHz as soon as their data lands. N=256 keeps
    # per-dummy granularity fine (~320ns cold) so the overshoot past DMA
    # arrival is small. All on zeroed scratch, off to the side.
    scr8 = persist.tile([P, 2, 256], FP8, tag="scr8")
    nc.gpsimd.memset(scr8, 0.0)
    scrf = persist.tile([P, 1], F32, tag="scrf")
    nc.gpsimd.memset(scrf, 0.0)
    dumo = persist.tile([P, 1], F32, tag="dumo")
    nc.scalar.activation(dumo, scrf, AF.Exp)

    ppw = psp.tile([P, 2 * NW], F32, tag="pp", name="pp_warm")
    for w in range(N_WARM):
        nc.tensor.matmul(
            ppw[:, 0:256],
            lhsT=scr8[:, :, 0:P],
            rhs=scr8,
            start=True,
            stop=True,
            perf_mode=DR,
        )

    # ---- main loop: 16 (g, m) slabs of [128 rows x 1024 cols], each one
    # 2-bank PSUM tile built by 8 DoubleRow matmuls, drained in-place by a
    # single Exp with fused row-sum accumulation. The first 12 accumulator
    # columns ship out early so only a tiny DMA trails the last drain.
    s_sb_a = persist.tile([P, 12], F32, tag="s_sb_a")
    s_sb_b = persist.tile([P, 4], F32, tag="s_sb_b")
    for g in range(G):
        for m in range(MT):
            pp = psp.tile([P, 2 * NW], F32, tag="pp", name=f"pp_{g}_{m}")
            for c in range(2):
                for t in range(KT2):
                    rhs = ags[g][:, 2 * t : 2 * t + 2, c * NW : (c + 1) * NW]
                    nc.tensor.matmul(
                        pp[:, c * NW : (c + 1) * NW],
                        lhsT=qms[m][:, 2 * t : 2 * t + 2, :],
                        rhs=rhs,
                        start=(t == 0),
                        stop=(t == KT2 - 1),
                        perf_mode=DR,
                    )
            col = g * MT + m
            acc = (
                s_sb_a[:, col : col + 1]
                if col < 12
                else s_sb_b[:, col - 12 : col - 11]
            )
            nc.scalar.activation(
                pp,
                pp,
                AF.Exp,
                scale=float(EXP_SCALE),
                accum_out=acc,
            )
        if g == G - 2:
            nc.sync.dma_start(out=s_out[:, 0:12], in_=s_sb_a)

    nc.sync.dma_start(out=s_out[:, 12:16], in_=s_sb_b)


_CACHE = {}


def _get_program():
    if "nc" not in _CACHE:
        _CACHE["nc"] = _build_program()
    return _CACHE["nc"]


def _make_in_maps(que, ans):
    """Normalize rows (folding the cosine norms into the quantization scale),
    quantize to fp8e4m3, and pack into the on-chip tile layouts. Also returns
    the exact host-computed diagonal logits."""
    fp8 = mybir.dt.np(FP8)
    que = np.asarray(que, dtype=np.float32)
    ans = np.asarray(ans, dtype=np.float32)

    qn = np.maximum(np.sqrt((que.astype(np.float64) ** 2).sum(1)), EPS)
    an = np.maximum(np.sqrt((ans.astype(np.float64) ** 2).sum(1)), EPS)
    q8 = (que * (SCALE / qn[:, None]).astype(np.float32)).astype(fp8)
    a8 = (ans * (SCALE / an[:, None]).astype(np.float32)).astype(fp8)

    # diag logits (exact, f64): cos(q_i, a_i) / gamma
    diag = (que.astype(np.float64) * ans.astype(np.float64)).sum(1) / (
        qn * an * GAMA
    )

    # aPK[g, p, 2t+i, j] = a8[1024g+j, 256t+128i+p]  (shared by all cores)
    aPK = np.ascontiguousarray(
        a8.reshape(G, 1024, KT2, 2, P).transpose(0, 4, 2, 3, 1)
    ).reshape(G, P, 2 * KT2, 1024)

    in_maps = []
    for c in range(NCORES):
        qc = q8[c * NB : (c + 1) * NB]  # [512, 1024]
        # qPK[m, p, 2t+i, mm] = qc[128m+mm, 256t+128i+p]
        qPK = np.ascontiguousarray(
            qc.reshape(MT, P, KT2, 2, P).transpose(0, 4, 2, 3, 1)
        ).reshape(MT, P, 2 * KT2, P)
        in_maps.append({"qPK": qPK, "aPK": aPK})
    return in_maps, diag


def _finish(results, diag):
    # s_out[p, 4g+m]: per-group partial softmax denominators.
    denoms = []
    for r in results:
        s = np.asarray(r["s_out"]).reshape(P, G, MT).sum(axis=1)  # [p, m]
        denoms.append(s.T.reshape(-1))  # local row order m*128+p
    denom = np.concatenate(denoms)  # [B]
    lse = np.log(denom.astype(np.float64))
    loss = np.float32(np.mean(lse - diag))
    return np.array([loss], dtype=np.float32)


def kernel(que_batch, ans_batch):
    nc = _get_program()
    in_maps, diag = _make_in_maps(np.asarray(que_batch), np.asarray(ans_batch))
    res = run_bass_kernel_spmd(nc, in_maps, list(range(NCORES)))
    return _finish(res.results, diag)


if __name__ == "__main__":
    rng = np.random.default_rng(0)
    q = rng.standard_normal((B, D), dtype=np.float32)
    a = rng.standard_normal((B, D), dtype=np.float32)
    print(kernel(q, a))
